# revision 2
# baseline (speedup 1.0000x reference)
"""Trainium2 Bass kernel for the de-stationary attention layer.

Problem shapes (hardcoded): B=4, L=S=2048, D=1024, H=16 heads, E=64.
  qh = q @ Wq.T + bq ; kh = k @ Wk.T + bk ; vh = v @ Wv.T + bv
  scores = qh @ kh.T (per head) * tau + delta ; a = softmax(scores / sqrt(E))
  out = (a @ vh) @ Wo.T + bo

Sharding: 8 cores = 4 batches x 2 head-groups (8 heads / 512 dims each).
Each core computes its batch's projections restricted to its head slice,
attention for its 8 heads, and a partial out-projection.  The host sums the
two partials per batch and adds bo.  tau/8 is folded into q (and bq) on the
host; delta/8 rides along as the per-partition bias of the exp activation.

On-chip layouts (per core; partition dim first):
  xq/xk/xv  [1024, 2048]  transposed inputs (D_in on partitions)
  qhT/khT   4 tiles [128, 2048]   o-dims on partitions (o-tile = head pair)
  vh        16 tiles [128, 8*65]  s on partitions; per head 64 dims + ones col
  p         [128, 1024]  exp(scoresT) per s-tile (s on partitions)
  pv psum   [65, 512]    rows 0-63 = attn out dims (transposed), row 64 = rowsum
  aT        4 tiles [128, 2048]   normalized attention, o on partitions
  out_part  [2048, 1024] partial out-projection (fp32)
"""

import sys

sys.path.insert(0, "/opt/trn_rl_repo")

import numpy as np
import ml_dtypes

import concourse.bass as bass
import concourse.mybir as mybir
import concourse.tile as tile
from concourse import bacc
from concourse.bass import ts
from concourse.bass_utils import run_bass_kernel_spmd

B, L, S, D, H, E = 4, 2048, 2048, 1024, 16, 64
NCORES = 8
GD = D // 2          # 512 head-dims per core
NOT = GD // 128      # 4 o-tiles per core (each = one head pair)
NST = S // 128       # 16 s-tiles
NLC = L // 512       # 4 l-chunks of 512
LSUP = 1024          # l-super span per exp call
NLS = L // LSUP      # 2

F32 = mybir.dt.float32
BF16 = mybir.dt.bfloat16
NPBF16 = ml_dtypes.bfloat16

# dtype knobs (bf16 matmuls with fp32 psum/exp/normalize)
DT_X = BF16      # transposed inputs in DRAM/SBUF
DT_W = BF16      # weights
DT_QK = BF16     # qhT / khT
DT_P = BF16      # exp(scores)
DT_V = BF16      # vh (+ones)
DT_A = BF16      # normalized attention (out-proj input)

_CACHE = {}


def _build_program():
    nc = bacc.Bacc("TRN2", target_bir_lowering=False, debug=False,
                   num_devices=NCORES)

    xq = nc.dram_tensor("xq", [D, L], DT_X, kind="ExternalInput")
    xk = nc.dram_tensor("xk", [D, S], DT_X, kind="ExternalInput")
    xv = nc.dram_tensor("xv", [D, S], DT_X, kind="ExternalInput")
    wq = nc.dram_tensor("wq", [D, GD], DT_W, kind="ExternalInput")
    wk = nc.dram_tensor("wk", [D, GD], DT_W, kind="ExternalInput")
    wv = nc.dram_tensor("wv", [D, GD], DT_W, kind="ExternalInput")
    wo = nc.dram_tensor("wo", [GD, D], DT_W, kind="ExternalInput")
    bqt = nc.dram_tensor("bqt", [128, NOT], F32, kind="ExternalInput")
    bkt = nc.dram_tensor("bkt", [128, NOT], F32, kind="ExternalInput")
    bvt = nc.dram_tensor("bvt", [128, GD], F32, kind="ExternalInput")
    d8 = nc.dram_tensor("d8", [128, NST], F32, kind="ExternalInput")
    out = nc.dram_tensor("out_part", [L, D], F32, kind="ExternalOutput")

    Exp = mybir.ActivationFunctionType.Exp

    with tile.TileContext(nc) as tc:
        from contextlib import ExitStack
        with ExitStack() as ctx:
            const = ctx.enter_context(tc.tile_pool(name="const", bufs=1))
            wpool = ctx.enter_context(tc.tile_pool(name="w", bufs=1))
            xpool = ctx.enter_context(tc.tile_pool(name="x", bufs=1))
            qk = ctx.enter_context(tc.tile_pool(name="qk", bufs=1))
            ppool = ctx.enter_context(tc.tile_pool(name="pp", bufs=3))
            npool = ctx.enter_context(tc.tile_pool(name="np", bufs=2))
            opool = ctx.enter_context(tc.tile_pool(name="op", bufs=4))
            psA = ctx.enter_context(tc.tile_pool(name="psA", bufs=2, space="PSUM"))
            psB = ctx.enter_context(tc.tile_pool(name="psB", bufs=2, space="PSUM"))

            # ---- constants ----
            bq_sb = const.tile([128, NOT], F32, tag="bq")
            nc.sync.dma_start(bq_sb[:], bqt.ap()[:])
            bk_sb = const.tile([128, NOT], F32, tag="bk")
            nc.sync.dma_start(bk_sb[:], bkt.ap()[:])
            bv_sb = const.tile([128, GD], F32, tag="bv")
            nc.sync.dma_start(bv_sb[:], bvt.ap()[:])
            d8_sb = const.tile([128, NST], F32, tag="d8")
            nc.sync.dma_start(d8_sb[:], d8.ap()[:])
            ones_sb = const.tile([128, 64], F32, tag="ones")
            nc.vector.memset(ones_sb[:], 1.0)

            wq_sb = [wpool.tile([128, GD], DT_W, tag=f"wq{i}", name=f"wq{i}") for i in range(8)]
            wk_sb = [wpool.tile([128, GD], DT_W, tag=f"wk{i}", name=f"wk{i}") for i in range(8)]
            wv_sb = [wpool.tile([128, GD], DT_W, tag=f"wv{i}", name=f"wv{i}") for i in range(8)]
            for i in range(8):
                nc.sync.dma_start(wq_sb[i][:], wq.ap()[ts(i, 128), :])
                nc.sync.dma_start(wk_sb[i][:], wk.ap()[ts(i, 128), :])
                nc.sync.dma_start(wv_sb[i][:], wv.ap()[ts(i, 128), :])
            wo_sb = [wpool.tile([128, D], DT_W, tag=f"wo{i}", name=f"wo{i}") for i in range(NOT)]
            for i in range(NOT):
                nc.sync.dma_start(wo_sb[i][:], wo.ap()[ts(i, 128), :])

            # ---- persistent activations ----
            qhT = [qk.tile([128, L], DT_QK, tag=f"qh{t}", name=f"qh{t}") for t in range(NOT)]
            khT = [qk.tile([128, S], DT_QK, tag=f"kh{t}", name=f"kh{t}") for t in range(NOT)]
            vh = [qk.tile([128, 8 * (E + 1)], DT_V, tag=f"vh{t}", name=f"vh{t}")
                  for t in range(NST)]
            aT = [qk.tile([128, L], DT_A, tag=f"aT{t}", name=f"aT{t}") for t in range(NOT)]

            # ---- phase V: vh[s_tile] = (v @ Wv.T + bv) per head + ones col ----
            xv_sb = [xpool.tile([128, S], DT_X, tag=f"x{i}", name=f"xt{i}") for i in range(8)]
            for i in range(8):
                nc.sync.dma_start(xv_sb[i][:], xv.ap()[ts(i, 128), :])
            bv3 = bv_sb.rearrange("p (h e) -> p h e", e=E)
            for t in range(NST):
                ps = psB.tile([128, GD], F32, tag="ps")
                for i in range(8):
                    nc.tensor.matmul(ps[:], lhsT=xv_sb[i][:, ts(t, 128)],
                                     rhs=wv_sb[i][:],
                                     start=(i == 0), stop=(i == 7))
                v3 = vh[t].rearrange("p (h w) -> p h w", w=E + 1)
                nc.vector.memset(v3[:, :, E:E + 1], 1.0)
                nc.vector.tensor_add(
                    v3[:, :, 0:E], ps.rearrange("p (h e) -> p h e", e=E), bv3)

            # ---- phase K then Q: khT/qhT[o_tile] = (x @ W.T + b).T ----
            for (xdram, w_sb, b_sb, dst) in ((xk, wk_sb, bk_sb, khT),
                                             (xq, wq_sb, bq_sb, qhT)):
                x_sb = [xpool.tile([128, S], DT_X, tag=f"x{i}", name=f"xt{i}") for i in range(8)]
                for i in range(8):
                    nc.sync.dma_start(x_sb[i][:], xdram.ap()[ts(i, 128), :])
                for ot in range(NOT):
                    for lc in range(NLC):
                        ps = psB.tile([128, 512], F32, tag="ps")
                        for i in range(8):
                            nc.tensor.matmul(ps[:],
                                             lhsT=w_sb[i][:, ts(ot, 128)],
                                             rhs=x_sb[i][:, ts(lc, 512)],
                                             start=(i == 0), stop=(i == 7))
                        nc.vector.tensor_scalar_add(
                            dst[ot][:, ts(lc, 512)], ps[:],
                            b_sb[:, ot:ot + 1])

            # ---- attention ----
            for h in range(8):
                ot, eo = h // 2, 64 * (h % 2)
                for ls in range(NLS):
                    pvs = [psB.tile([128, 512], F32, tag=f"pv{j}", bufs=1, name=f"pv{j}")
                           for j in range(2)]
                    for t in range(NST):
                        sc = psA.tile([128, LSUP], F32, tag="sc")
                        for j in range(2):
                            nc.tensor.matmul(
                                sc[:, ts(j, 512)],
                                lhsT=khT[ot][eo:eo + 64, ts(t, 128)],
                                rhs=qhT[ot][eo:eo + 64,
                                            ls * LSUP + 512 * j:
                                            ls * LSUP + 512 * (j + 1)],
                                start=True, stop=True)
                        p_t = ppool.tile([128, LSUP], DT_P, tag="p")
                        nc.scalar.activation(p_t[:], sc[:], Exp,
                                             bias=d8_sb[:, t:t + 1])
                        for j in range(2):
                            nc.tensor.matmul(
                                pvs[j][0:E + 1, :],
                                lhsT=vh[t][:, (E + 1) * h:(E + 1) * (h + 1)],
                                rhs=p_t[:, ts(j, 512)],
                                start=(t == 0), stop=(t == NST - 1))
                    # normalize: aT[o, l] = pv[o, l] / pv[64, l]
                    for j in range(2):
                        loff = ls * LSUP + 512 * j
                        rs = npool.tile([128, 512], F32, tag="rs")
                        nc.vector.reciprocal(rs[64:65, :], pvs[j][E:E + 1, :])
                        bc = psB.tile([128, 512], F32, tag="ps")
                        nc.tensor.matmul(bc[0:64, :], lhsT=ones_sb[64:65, :],
                                         rhs=rs[64:65, :],
                                         start=True, stop=True)
                        bcs = npool.tile([128, 512], F32, tag="bcs")
                        nc.vector.tensor_copy(bcs[0:64, :], bc[0:64, :])
                        if eo == 0:
                            nc.vector.tensor_mul(
                                aT[ot][0:64, loff:loff + 512],
                                pvs[j][0:64, :], bcs[0:64, :])
                        else:
                            stg = npool.tile([128, 512], DT_A, tag="stg")
                            nc.vector.tensor_mul(
                                stg[0:64, :], pvs[j][0:64, :], bcs[0:64, :])
                            nc.sync.dma_start(
                                aT[ot][64:128, loff:loff + 512], stg[0:64, :])

            # ---- out-projection: out[l, dm] = sum_o aT[o, l] * woT[o, dm] ----
            for lt in range(L // 128):
                for nh in range(2):
                    ps = psB.tile([128, 512], F32, tag="ps")
                    for ot in range(NOT):
                        nc.tensor.matmul(ps[:], lhsT=aT[ot][:, ts(lt, 128)],
                                         rhs=wo_sb[ot][:, ts(nh, 512)],
                                         start=(ot == 0), stop=(ot == NOT - 1))
                    osb = opool.tile([128, 512], F32, tag="ot")
                    nc.vector.tensor_copy(osb[:], ps[:])
                    nc.sync.dma_start(out.ap()[ts(lt, 128), ts(nh, 512)],
                                      osb[:])

    nc.compile()
    return nc


def _prep_inputs(q, k, v, tau, delta, Wq, bq, Wk, bk, Wv, bv, Wo, bo):
    """Build the per-core input maps (host-side shard + transpose + cast)."""
    in_maps = []
    q = np.asarray(q, np.float32)
    k = np.asarray(k, np.float32)
    v = np.asarray(v, np.float32)
    for c in range(NCORES):
        b, g = c // 2, c % 2
        gsl = slice(g * GD, (g + 1) * GD)
        t8 = np.float32(np.asarray(tau, np.float32)[b, 0] / 8.0)
        m = {
            "xq": np.ascontiguousarray((q[b].T * t8).astype(NPBF16)),
            "xk": np.ascontiguousarray(k[b].T.astype(NPBF16)),
            "xv": np.ascontiguousarray(v[b].T.astype(NPBF16)),
            "wq": np.ascontiguousarray(np.asarray(Wq, np.float32)[gsl, :].T
                                       .astype(NPBF16)),
            "wk": np.ascontiguousarray(np.asarray(Wk, np.float32)[gsl, :].T
                                       .astype(NPBF16)),
            "wv": np.ascontiguousarray(np.asarray(Wv, np.float32)[gsl, :].T
                                       .astype(NPBF16)),
            "wo": np.ascontiguousarray(np.asarray(Wo, np.float32)[:, gsl].T
                                       .astype(NPBF16)),
            "bqt": np.ascontiguousarray(
                (np.asarray(bq, np.float32)[gsl] * t8).reshape(NOT, 128).T),
            "bkt": np.ascontiguousarray(
                np.asarray(bk, np.float32)[gsl].reshape(NOT, 128).T),
            "bvt": np.ascontiguousarray(
                np.tile(np.asarray(bv, np.float32)[gsl], (128, 1))),
            "d8": np.ascontiguousarray(
                (np.asarray(delta, np.float32)[b] / 8.0).reshape(NST, 128).T),
        }
        in_maps.append(m)
    return in_maps


def _get_nc():
    if "nc" not in _CACHE:
        _CACHE["nc"] = _build_program()
    return _CACHE["nc"]


def kernel(q, k, v, tau, delta, Wq, bq, Wk, bk, Wv, bv, Wo, bo):
    nc = _get_nc()
    in_maps = _prep_inputs(q, k, v, tau, delta, Wq, bq, Wk, bk, Wv, bv, Wo, bo)
    res = run_bass_kernel_spmd(nc, in_maps, list(range(NCORES)))
    bo = np.asarray(bo, np.float32)
    out = np.empty((B, L, D), np.float32)
    for b in range(B):
        out[b] = (res.results[2 * b]["out_part"]
                  + res.results[2 * b + 1]["out_part"] + bo)
    return out
